# revision 12
# baseline (speedup 1.0000x reference)
"""Trainium2 Bass kernel for nn_Encoder_Postnet_combine (B=16,T=4096,P=512,D=512,S=100).

Math (algebraically folded from the reference):
  idx[b,t]   : sequential aligner scan (host, tiny integer recurrence)
  W1 = w_out[:D]; W2 = w_out[D:]
  Wc  = (I + w_pos) @ W1
  EW  = encoder_out @ Wc                       (device GEMM, per batch)
  v   = w_pitch[0] @ W1
  dEb = (emb_beats[1]-emb_beats[0]) @ W1
  EsW = emb_singer @ W2
  PEW = pe @ (w_pos @ W1) + (b_pitch+b_pos+emb_beats[0]) @ W1 + b_out
  out = leaky( EW[b,idx] + EsW[sv] + PEW[t] + pitch*v + beats*dEb , 0.01)

Device strategy (per core, data-parallel over batch, 2 batches/core):
  Everything in phase B is PE matmuls accumulating into PSUM:
    - EW[idx] expansion: one-hot matrix G (baked from host-computed idx)
      times a 16..32-row slice of SBUF-resident EW.
    - EsW[sv] + beats*dEb + pitch*v: a 102-row "S" matmul whose lhsT holds
      per-position singer one-hots, the beats value and the pitch value
      (host-baked blob); rhs is the static table [EsW; dEb; v].
    - PEW[t]: identity matmul against SBUF-resident PEW tiles.
  Then one batched leaky-ReLU on the scalar engine reads PSUM pairs and a
  grouped DMA stores 4 tiles at once. All device data is bf16 (fp32 PSUM).
"""
import numpy as np
import ml_dtypes

import concourse.bass as bass
import concourse.mybir as mybir
import concourse.tile as tile
from concourse.vector_clock import ScopedClock
from concourse.bass_utils import run_bass_kernel_spmd

F32 = mybir.dt.float32
BF16 = mybir.dt.bfloat16

B, T, PH, D, S = 16, 4096, 512, 512, 100
NCORES = 8
BPC = B // NCORES          # batches per core
TT = T // 128              # 32 t-tiles per batch
NT = BPC * TT              # 64 out-tiles per core

SROWS = 102                # singer one-hot (100) + beats row + pitch row

# ---------------------------------------------------------------------------
# Workarounds for this walrus build: at most ONE sync wait per instruction
# (EventSemaphore: 2).


def _split_drain_and_barrier(self, tick_clock, wait_clock):
    nc = self.nc
    probe = nc.sync.nop()
    wait_clock.add_sem_waits(probe.ins, ScopedClock({None: tick_clock.global_clock}))
    si = probe.ins.sync_info
    if si is not None and si.on_wait and len(si.on_wait) > 1:
        waits = list(si.on_wait)
        si.on_wait = waits[:1]
        for w in waits[1:]:
            extra = nc.sync.nop()
            extra.ins.sync_info = mybir.SyncInfo(on_wait=[w], on_update=[])
    nc.sync.drain()
    nc.all_engine_barrier()
    assert self.sems is not None
    popped = nc._tile_sem_poison_stack.pop()
    assert popped is self._sem_poison
    nc.clear_and_free_semaphores(list(self.sems.allocated().values()))
    nc.all_engine_barrier()


tile.TileContext._drain_and_barrier = _split_drain_and_barrier


def _split_multi_waits(nc):
    counter = [0]

    def fresh_nop(engine, wait):
        counter[0] += 1
        nop = mybir.InstNoOp(name=f"waitsplit_{counter[0]}", ins=[], outs=[])
        nop.engine = engine
        nop.sync_info = mybir.SyncInfo(on_wait=[wait], on_update=[])
        return nop

    for fn in nc.m.functions:
        for blk in fn.blocks:
            new_insts = []
            for inst in blk.instructions:
                si = inst.sync_info
                limit = 2 if isinstance(inst, mybir.InstEventSemaphore) else 1
                if si is not None and si.on_wait and len(si.on_wait) > limit:
                    waits = list(si.on_wait)
                    for w in waits[:-limit]:
                        new_insts.append(fresh_nop(inst.engine, w))
                    si.on_wait = waits[-limit:]
                new_insts.append(inst)
            blk.instructions = new_insts


# ---------------------------------------------------------------------------
# Device program. `segs[n]` = list of (chunk, part_off, length, blob_row_off)
# describing the EW rows out-tile n gathers (baked from host idx).


def build_program(segs, repeat=1):
    nc = bass.Bass()
    encT = nc.declare_dram_parameter("encT", [BPC * D, PH], BF16, isOutput=False)
    wc = nc.declare_dram_parameter("wc", [D, D], BF16, isOutput=False)
    pew = nc.declare_dram_parameter("pew", [T, D], BF16, isOutput=False)
    stat = nc.declare_dram_parameter("stat", [128, D], BF16, isOutput=False)
    blob = nc.declare_dram_parameter("blob", [128, NT * 128], BF16, isOutput=False)
    gblob = nc.declare_dram_parameter("gblob", [128, NT * 128], BF16, isOutput=False)
    ident = nc.declare_dram_parameter("ident", [128, 128], BF16, isOutput=False)
    out = nc.declare_dram_parameter("out", [BPC * T, D], BF16, isOutput=True)

    with tile.TileContext(nc) as tc:
        with (
            tc.tile_pool(name="const", bufs=1) as cpool,
            tc.tile_pool(name="sbuf", bufs=3) as pool,
            tc.tile_pool(name="psA", bufs=2, space="PSUM") as psumA,
            tc.tile_pool(name="psB", bufs=3, space="PSUM") as psum,
        ):
            def body(_=None):
                # --- constants / small inputs ---
                wc_sb = cpool.tile([128, 4, D], BF16, tag="wc")
                nc.sync.dma_start(
                    out=wc_sb[:], in_=wc[:].rearrange("(j p) d -> p j d", p=128))
                stat_sb = cpool.tile([128, D], BF16, tag="stat")
                nc.sync.dma_start(out=stat_sb[:], in_=stat[:])
                ident_sb = cpool.tile([128, 128], BF16, tag="ident")
                nc.sync.dma_start(out=ident_sb[:], in_=ident[:])
                enc_sb = []
                for b in range(BPC):
                    e_t = cpool.tile([128, 4, PH], BF16, tag=f"encT{b}")
                    nc.sync.dma_start(
                        out=e_t[:],
                        in_=encT[b * D:(b + 1) * D, :].rearrange(
                            "(j p) d -> p j d", p=128))
                    enc_sb.append(e_t)
                blob_sb = cpool.tile([128, NT * 128], BF16, tag="blob")
                for q in range(4):
                    c0 = q * (NT * 32)
                    nc.sync.dma_start(out=blob_sb[:, c0:c0 + NT * 32],
                                      in_=blob[:, c0:c0 + NT * 32])
                gblob_sb = cpool.tile([128, NT * 128], BF16, tag="gblob")
                for q in range(4):
                    c0 = q * (NT * 32)
                    nc.sync.dma_start(out=gblob_sb[:, c0:c0 + NT * 32],
                                      in_=gblob[:, c0:c0 + NT * 32])
                pew_sb = cpool.tile([128, TT, D], BF16, tag="pew")
                for q in range(8):
                    r0 = q * 4 * 128
                    nc.sync.dma_start(
                        out=pew_sb[:, q * 4:(q + 1) * 4, :],
                        in_=pew[r0:r0 + 512, :].rearrange("(g p) d -> p g d", p=128))

                # --- phase A: EW = E @ Wc per batch -> SBUF (bf16) ---
                ew_sb = []
                for b in range(BPC):
                    for mm in range(4):
                        ps = psumA.tile([128, D], F32, tag="ps_ew")
                        for ki in range(4):
                            nc.tensor.matmul(
                                out=ps[:],
                                lhsT=enc_sb[b][:, ki, mm * 128:(mm + 1) * 128],
                                rhs=wc_sb[:, ki, :],
                                start=(ki == 0),
                                stop=(ki == 3),
                            )
                        ew_t = cpool.tile([128, D], BF16, tag=f"ew{b}_{mm}")
                        nc.vector.tensor_copy(out=ew_t[:], in_=ps[:])
                        ew_sb.append(ew_t)

                # --- phase B: all-PE accumulation + batched leaky + stores ---
                for b in range(BPC):
                    for tp in range(TT // 2):
                        ps = psum.tile([128, 2 * D], F32, tag="ps")
                        for h in (0, 1):
                            tt = 2 * tp + h
                            n = b * TT + tt
                            o = ps[:, h * D:(h + 1) * D]
                            cols = slice(n * 128, (n + 1) * 128)
                            for si, (c, base, ln) in enumerate(segs[n]):
                                nc.tensor.matmul(
                                    out=o,
                                    lhsT=gblob_sb[base:base + ln, cols],
                                    rhs=ew_sb[b * 4 + c][base:base + ln, :],
                                    start=(si == 0), stop=False)
                            nc.tensor.matmul(
                                out=o,
                                lhsT=blob_sb[0:SROWS, cols],
                                rhs=stat_sb[0:SROWS, :],
                                start=False, stop=False)
                            nc.tensor.matmul(
                                out=o,
                                lhsT=ident_sb[:],
                                rhs=pew_sb[:, tt, :],
                                start=False, stop=True)
                        j = tp % 2
                        if j == 0:
                            stage = pool.tile([128, 4, D], BF16, tag="stage")
                        nc.scalar.activation(
                            out=stage[:, 2 * j:2 * j + 2, :], in_=ps[:],
                            func=mybir.ActivationFunctionType.Lrelu, alpha=0.01)
                        if j == 1:
                            r0 = b * T + (tp - 1) * 2 * 128
                            nc.gpsimd.dma_start(
                                out=out[r0:r0 + 512, :].rearrange(
                                    "(g p) d -> p g d", p=128),
                                in_=stage[:])

            if repeat == 1:
                body()
            else:
                for _ in range(repeat):
                    body()

    _split_multi_waits(nc)
    return nc


# ---------------------------------------------------------------------------
# Host side


def _host_scan_idx(align, text):
    align = np.asarray(align, dtype=np.int64)
    text = np.asarray(text, dtype=np.int64)
    Bn, Tn = align.shape
    Pn = text.shape[1]
    idx = np.zeros((Bn, Tn), dtype=np.int32)
    ind = np.zeros(Bn, dtype=np.int64)
    rows = np.arange(Bn)
    cur = text[rows, ind]
    for t in range(1, Tn):
        a = align[:, t]
        stay = a == cur
        ind = np.where(stay, ind, np.minimum(ind + 1, Pn - 1))
        cur = np.where(stay, cur, text[rows, ind])
        idx[:, t] = ind
    return idx


def _positional_encoding(length, d_model):
    pos = np.arange(length, dtype=np.float32)[:, None]
    div = np.exp(np.arange(0, d_model, 2, dtype=np.float32)
                 * (-np.log(10000.0) / d_model))
    pe = np.zeros((length, d_model), np.float32)
    pe[:, 0::2] = np.sin(pos * div)
    pe[:, 1::2] = np.cos(pos * div)
    return pe


def _fold(w_pitch, b_pitch, w_pos, b_pos, emb_beats, emb_singer, w_out, b_out):
    f64 = np.float64
    W1 = np.asarray(w_out[:D], f64)
    W2 = np.asarray(w_out[D:], f64)
    WposW1 = np.asarray(w_pos, f64) @ W1
    Wc = (W1 + WposW1).astype(np.float32)
    v = (np.asarray(w_pitch[0], f64) @ W1).astype(np.float32)
    EbW = np.asarray(emb_beats, f64) @ W1
    dEb = (EbW[1] - EbW[0]).astype(np.float32)
    EsW = (np.asarray(emb_singer, f64) @ W2).astype(np.float32)
    cb = (np.asarray(b_pitch + b_pos, f64) @ W1 + EbW[0] + np.asarray(b_out, f64))
    pe = _positional_encoding(T, D)
    PEW = (np.asarray(pe, f64) @ WposW1 + cb[None, :]).astype(np.float32)
    return Wc, v, dEb, EsW, PEW


def _bf16(x):
    return np.ascontiguousarray(np.asarray(x, np.float32).astype(ml_dtypes.bfloat16))


_CACHE = {}


def kernel(encoder_out, align_phone, text_phone, pitch, beats, singer_vec,
           w_pitch, b_pitch, w_pos, b_pos, emb_beats, emb_singer, w_out, b_out):
    encoder_out = np.ascontiguousarray(np.asarray(encoder_out, np.float32))
    pitch = np.asarray(pitch, np.float32)[..., 0]                    # [B,T]
    beats_f = np.asarray(beats, np.int64)[..., 0].astype(np.float32)  # [B,T]
    sv = np.asarray(singer_vec, np.int64)[..., 0].astype(np.int32)    # [B,T]

    idx = _host_scan_idx(align_phone, text_phone)          # [B,T] int32
    Wc, v, dEb, EsW, PEW = _fold(
        np.asarray(w_pitch, np.float32), np.asarray(b_pitch, np.float32),
        np.asarray(w_pos, np.float32), np.asarray(b_pos, np.float32),
        np.asarray(emb_beats, np.float32), np.asarray(emb_singer, np.float32),
        np.asarray(w_out, np.float32), np.asarray(b_out, np.float32))

    # Per-out-tile idx windows, shared across cores (baked into the program).
    # PE tile_position: K<=32 slices may start at base 0/32/64/96, K<=64 at
    # 0/64, else 0; lhsT and rhs bases must match.
    idx_n = idx.reshape(NCORES, BPC * TT, 128)             # [core, n, q]
    lo = idx_n.min(axis=0).min(axis=1)                     # [NT]
    hi = idx_n.max(axis=0).max(axis=1)                     # [NT]
    segs = []
    for n in range(NT):
        s, e = int(lo[n]), int(hi[n]) + 1
        parts, used = [], []
        while s < e:
            c = s // 128
            s_in, e_in = s - c * 128, min(e - c * 128, 128)
            # Allowed base partitions are {0, 32, 64} (K<=32 for base 32).
            base = min(s_in - s_in % 32, 64)
            ln = e_in - base
            if not (ln <= 32 or (base in (0, 64) and ln <= 64)):
                base = min(s_in - s_in % 64, 64)
                ln = e_in - base
                if ln > 64:
                    base, ln = 0, e_in
            for b0, b1 in used:
                assert b1 <= base or base + ln <= b0, (
                    f"G row ranges overlap for out-tile {n}")
            used.append((base, base + ln))
            parts.append((c, base, ln))
            s = c * 128 + e_in
        segs.append(parts)

    # Static rhs table: [EsW(100); dEb; v; 0...]
    stat = np.zeros((128, D), np.float32)
    stat[:S] = EsW
    stat[S] = dEb
    stat[S + 1] = v

    if "nc" not in _CACHE or _CACHE.get("segs") != segs:
        _CACHE["nc"] = build_program(segs)
        _CACHE["segs"] = segs

    nq = np.arange(NT)[:, None]
    qq = np.arange(128)[None, :]
    in_maps = []
    for core in range(NCORES):
        b0 = core * BPC
        sl = slice(b0, b0 + BPC)
        encT = np.ascontiguousarray(
            encoder_out[sl].transpose(0, 2, 1).reshape(BPC * D, PH))
        # Blob [128, NT, 128]: singer one-hot rows, beats row, pitch row, G rows.
        blob = np.zeros((128, NT, 128), np.float32)
        svn = sv[sl].reshape(NT, 128)
        blob[svn, nq, qq] = 1.0
        blob[S, :, :] = beats_f[sl].reshape(NT, 128)
        blob[S + 1, :, :] = pitch[sl].reshape(NT, 128)
        gb = np.zeros((128, NT, 128), np.float32)
        gb[idx[sl].reshape(NT, 128) % 128, nq, qq] = 1.0
        in_maps.append({
            "encT": _bf16(encT),
            "wc": _bf16(Wc),
            "pew": _bf16(PEW),
            "stat": _bf16(stat),
            "blob": _bf16(blob.reshape(128, NT * 128)),
            "gblob": _bf16(gb.reshape(128, NT * 128)),
            "ident": _bf16(np.eye(128, dtype=np.float32)),
        })

    _CACHE["last_in_maps"] = in_maps
    res = run_bass_kernel_spmd(_CACHE["nc"], in_maps, core_ids=list(range(NCORES)))
    out = np.empty((B, T, D), np.float32)
    for c in range(NCORES):
        out[c * BPC:(c + 1) * BPC] = (
            res.results[c]["out"].astype(np.float32).reshape(BPC, T, D))
    return out
